# revision 5
# baseline (speedup 1.0000x reference)
"""Trainium2 Bass kernel for nn_CovarianceLayer (Toeplitz-autocorrelation form).

Math: x = inputs[:,0,:] + i*inputs[:,1,:]  (B=256 complex signals, N=1024)
      cov[b,l,m] = Re(hankel @ hankel^H)[l,m] / L  with hankel[b,i,j] = x[b,(j+i)%N]
By circularity cov[b,l,m] = r_b[|l-m|] / L where
      r_b[d] = sum_n ( xr[n]xr[n+d] + xi[n]xi[n+d] )   (indices mod N)
i.e. each [L,L] output tile is a symmetric Toeplitz matrix fully
determined by a 128-lag autocorrelation r_b, so only r_b is computed.

Per-core pipeline (32 batches/core, pure data parallel):
  1. Casting DMAs build a doubled fp8e4m3 copy of x in DRAM (row per
     (batch,comp): [x | x]; the duplication realizes the circular wrap).
     Tapered cast groups: a tiny first cast unblocks the first Hankel
     tile early.
  2. One merged HWDGE DMA per supergroup builds a packed Hankel tile
     H[16c+p, j*W+u] = x_c[b_j, p+u] (4-dim overlapping-window access
     pattern covering both complex components).
  3. 32 DoubleRow fp8 matmuls per batch (each contracts 64 products)
     accumulate r_b into one psum column.
  4. Per compute group: DVE drains psum -> SBUF (f32r), then ONE f32r
     matmul against a constant palindrome matrix (spal[d,k] =
     [d==|k-127|]/L) applies transpose + mirror + 1/L in one PE op.
  5. One strided DMA per group expands the Toeplitz tiles straight from
     the palindrome PSUM tile into the output: out[b,l,m] = pt2[b,
     127-l+m] (contiguous 512B runs on both sides).
Supergroups and compute groups taper up ([2,...]) so the first
expansion DMA starts as early as possible; after that the DMA engines
(the shared bottleneck device) stay continuously busy draining
expansions while later groups' matmuls run ahead on PE.
"""

import numpy as np

import concourse.bacc as bacc
import concourse.mybir as mybir
import concourse.tile as tile
from concourse.bass_types import AP
from concourse.bass_utils import run_bass_kernel_spmd

B, L, N = 256, 128, 1024
NCORES = 8
BPC = B // NCORES  # 32 batches per core

P = 16  # n-offsets per chunk
K = 2 * P  # matmul contraction width
T = N // P  # 64 chunks per batch
W = N - P + 128  # 1136
CROW = 2 * N  # doubled per-comp row in xdup
ROW = 2 * CROW  # 4096 elems per batch

_CACHE = {}
LAST_RESULT = None

SGROUPS = [6, 10, 8, 8]  # hankel supergroup sizes
CGROUPS = [32]  # cast group sizes
GROUPS = [2, 4, 5, 7, 7, 7]  # compute/expansion group sizes


def build_nc(sgroups=None, groups=None, cgroups=None, fin_at=1,
             pbufs=(4, 4), psum_src=False, split_hankel=True,
             cast_eng=("gpsimd",), hank_eng=None, spal_eng="scalar",
             exp_eng=None):
    f8 = mybir.dt.float8e4
    f32 = mybir.dt.float32
    f32r = mybir.dt.float32r
    sgroups = list(SGROUPS if sgroups is None else sgroups)
    groups = list(GROUPS if groups is None else groups)
    cgs = list(CGROUPS if cgroups is None else cgroups)
    assert sum(sgroups) == BPC and sum(groups) == BPC and sum(cgs) == BPC
    sstart = [sum(sgroups[:i]) for i in range(len(sgroups))]
    gstart = [sum(groups[:i]) for i in range(len(groups))]
    cstart = [sum(cgs[:i]) for i in range(len(cgs))]
    ng = len(groups)
    if hank_eng is None:
        hank_eng = ["scalar"] * len(sgroups)
    if exp_eng is None:
        exp_eng = ["sync"] * ng
    # map batch -> supergroup index
    b2s = []
    for s, sz in enumerate(sgroups):
        b2s += [s] * sz

    nc = bacc.Bacc(
        "TRN2", target_bir_lowering=False, debug=False, num_devices=NCORES
    )
    inp = nc.dram_tensor("inp", [BPC, 2, N], f32, kind="ExternalInput")
    spald = nc.dram_tensor("spald", [128, 256], f32r, kind="ExternalInput")
    out = nc.dram_tensor("out", [BPC, L, L], f32, kind="ExternalOutput")

    def eng(name):
        return getattr(nc, name)

    with tile.TileContext(nc) as tc:
        with (
            tc.tile_pool(name="const", bufs=1) as cpool,
            tc.tile_pool(name="dram", bufs=1, space="DRAM") as dpool,
            tc.tile_pool(name="hank", bufs=len(sgroups)) as hpool,
            tc.tile_pool(name="spal", bufs=ng) as spool,
            tc.tile_pool(name="rr", bufs=ng) as rpool,
            tc.tile_pool(name="psum", bufs=pbufs[0], space="PSUM") as ppool,
            tc.tile_pool(name="pst", bufs=pbufs[1], space="PSUM") as tpool,
        ):
            # --- doubled fp8 signal in DRAM via tapered casting DMAs,
            # emitted FIRST. Each cast group gets its own DRAM tile so
            # hankel DMAs depend only on their own cast. ---
            flat = inp[:].rearrange("b c n -> (b c) n")
            xdups = []  # one per cast group
            for si, ssz in enumerate(cgs):
                xd = dpool.tile([2 * ssz, CROW], f8)
                xdups.append(xd)
                r0 = 2 * cstart[si]
                src0 = AP(
                    tensor=flat.tensor,
                    offset=flat.offset + r0 * N,
                    ap=[[N, 2 * ssz], [0, 2], [1, N]],
                )
                dst0 = AP(
                    tensor=xd.tensor,
                    offset=xd.offset,
                    ap=[[CROW, 2 * ssz], [N, 2], [1, N]],
                )
                eng(cast_eng[min(si, len(cast_eng) - 1)]).dma_start(
                    out=dst0, in_=src0
                )

            # --- palindrome matrix for the finish matmul ---
            spal_t = cpool.tile([128, 256], f32r)
            eng(spal_eng).dma_start(out=spal_t[:], in_=spald[:])

            # --- hankel tiles: H[16c+p, j*W+u] = x_c[b_j, p+u] ---
            htiles = []
            for s, ssz in enumerate(sgroups):
                ht = hpool.tile([K, ssz * W], f8)
                htiles.append(ht)
                ci = max(i for i in range(len(cgs)) if cstart[i] <= sstart[s])
                assert cstart[ci] + cgs[ci] >= sstart[s] + ssz, "sg spans casts"
                xd = xdups[ci]
                xoff = 2 * (sstart[s] - cstart[ci]) * CROW
                W_ = ssz * W
                if split_hankel:
                    for c in range(2):
                        src = AP(
                            tensor=xd.tensor,
                            offset=xd.offset + xoff + c * CROW,
                            ap=[[1, P], [ROW, ssz], [1, W]],
                        )
                        eng(hank_eng[s]).dma_start(
                            out=ht[P * c : P * c + P, :], in_=src
                        )
                else:
                    src = AP(
                        tensor=xd.tensor,
                        offset=xd.offset + xoff,
                        ap=[[CROW, 2], [1, P], [ROW, ssz], [1, W]],
                    )
                    dst = AP(
                        tensor=ht.tensor,
                        offset=ht.offset,
                        ap=[[P * W_, 2], [W_, P], [W, ssz], [1, W]],
                    )
                    eng(hank_eng[s]).dma_start(out=dst, in_=src)

            rgs = {}

            def finish(g):
                gb = groups[g]
                # one matmul applies transpose+mirror+1/L:
                # pt2[j,k] = sum_d rg[d,j]*spal[d,k],  spal[d,k]=[d==|k-127|]/L
                pt2 = tpool.tile([gb, 256], f32)
                nc.tensor.matmul(pt2[:], rgs[g][:], spal_t[:])
                if psum_src:
                    src2 = AP(
                        tensor=pt2.tensor,
                        offset=pt2.offset + 127,
                        ap=[[256, gb], [-1, 128], [1, 128]],
                    )
                else:
                    rows = spool.tile([gb, 256], f32)
                    nc.vector.tensor_copy(rows[:, 0:255], pt2[:, 0:255])
                    src2 = AP(
                        tensor=rows.tensor,
                        offset=rows.offset + 127,
                        ap=[[256, gb], [-1, 128], [1, 128]],
                    )
                dst2 = AP(
                    tensor=out,
                    offset=gstart[g] * L * L,
                    ap=[[L * L, gb], [L, 128], [1, 128]],
                )
                eng(exp_eng[g]).dma_start(out=dst2, in_=src2)

            for g in range(ng):
                gb = groups[g]
                ps = ppool.tile([128, gb], f32)
                fa = fin_at if fin_at is not None else max(1, gb // 2)
                for j in range(gb):
                    if j == min(fa, gb - 1) and g >= 1:
                        finish(g - 1)
                    b = gstart[g] + j
                    s = b2s[b]
                    ht = htiles[s]
                    col = (b - sstart[s]) * W
                    for tp in range(T // 2):
                        off = col + K * tp
                        lhsT = AP(
                            tensor=ht.tensor,
                            offset=ht.offset + off,
                            ap=[[sgroups[s] * W, K], [P, 2], [1, 128]],
                        )
                        rhs = AP(
                            tensor=ht.tensor,
                            offset=ht.offset + off,
                            ap=[[sgroups[s] * W, K], [P, 2], [1, 1]],
                        )
                        nc.tensor.matmul(
                            ps[:, j : j + 1],
                            lhsT,
                            rhs,
                            start=(tp == 0),
                            stop=(tp == T // 2 - 1),
                            perf_mode=mybir.MatmulPerfMode.DoubleRow,
                        )

                rg = rpool.tile([128, gb], f32r)
                nc.vector.tensor_copy(rg[:], ps[:])
                rgs[g] = rg
            finish(ng - 1)

    nc.compile()
    return nc


def kernel(inputs: np.ndarray) -> np.ndarray:
    global LAST_RESULT
    inputs = np.ascontiguousarray(np.asarray(inputs), dtype=np.float32)
    assert inputs.shape == (B, 2, N), inputs.shape

    if "nc" not in _CACHE:
        _CACHE["nc"] = build_nc()
    nc = _CACHE["nc"]

    k = np.arange(256)
    d = np.arange(128)
    spal = (d[:, None] == np.minimum(np.abs(k[None, :] - 127), 127)).astype(
        np.float32
    ) / L
    spal[:, 255] = 0.0
    in_maps = [
        {"inp": inputs[c * BPC : (c + 1) * BPC], "spald": spal}
        for c in range(NCORES)
    ]
    res = run_bass_kernel_spmd(nc, in_maps, list(range(NCORES)), trace=False)
    LAST_RESULT = res
    outf = np.concatenate([res.results[c]["out"] for c in range(NCORES)], axis=0)
    return outf.reshape(B, L, L, 1).astype(np.float32, copy=False)


# revision 14
# speedup vs baseline: 1.0524x; 1.0524x over previous
"""Trainium2 Bass kernel for nn_CovarianceLayer (Toeplitz-autocorrelation form).

Math: x = inputs[:,0,:] + i*inputs[:,1,:]  (B=256 complex signals, N=1024)
      cov[b,l,m] = Re(hankel @ hankel^H)[l,m] / L  with hankel[b,i,j] = x[b,(j+i)%N]
By circularity cov[b,l,m] = r_b[|l-m|] / L where
      r_b[d] = sum_n ( xr[n]xr[n+d] + xi[n]xi[n+d] )   (indices mod N)
i.e. each [L,L] output tile is a symmetric Toeplitz matrix fully
determined by a 128-lag autocorrelation r_b, so only r_b is computed.

Per-core pipeline (32 batches/core, pure data parallel):
  1. Casting DMAs build a doubled fp8e4m3 copy of x in DRAM (row per
     (batch,comp): [x | x]; the duplication realizes the circular wrap).
     Tapered cast groups: a tiny first cast unblocks the first Hankel
     tile early.
  2. One merged HWDGE DMA per supergroup builds a packed Hankel tile
     H[16c+p, j*W+u] = x_c[b_j, p+u] (4-dim overlapping-window access
     pattern covering both complex components).
  3. 32 DoubleRow fp8 matmuls per batch (each contracts 64 products)
     accumulate r_b into one psum column.
  4. Per compute group: DVE drains psum -> SBUF (f32r), then ONE f32r
     matmul against a constant palindrome matrix (spal[d,k] =
     [d==|k-127|]/L) applies transpose + mirror + 1/L in one PE op.
  5. One strided DMA per group expands the Toeplitz tiles straight from
     the palindrome PSUM tile into the output: out[b,l,m] = pt2[b,
     127-l+m] (contiguous 512B runs on both sides).
Supergroups and compute groups taper up ([2,...]) so the first
expansion DMA starts as early as possible; after that the DMA engines
(the shared bottleneck device) stay continuously busy draining
expansions while later groups' matmuls run ahead on PE.
"""

import numpy as np

import concourse.bacc as bacc
import concourse.mybir as mybir
import concourse.tile as tile
from concourse.bass_types import AP
from concourse.bass_utils import run_bass_kernel_spmd

B, L, N = 256, 128, 1024
NCORES = 8
BPC = B // NCORES  # 32 batches per core

P = 16  # n-offsets per chunk
K = 2 * P  # matmul contraction width
T = N // P  # 64 chunks per batch
W = N - P + 128  # 1136
CROW = 2 * N  # doubled per-comp row in xdup
ROW = 2 * CROW  # 4096 elems per batch

_CACHE = {}
LAST_RESULT = None

SGROUPS = [8, 8, 8, 8]  # hankel supergroup sizes
CGROUPS = [8, 8, 8, 8]  # cast group sizes
GROUPS = [6, 6, 5, 6, 4, 2, 3]  # compute/expansion group sizes


DXN = 1152  # new-layout per-comp row: [x | x[0:128]]
BWN = 318  # new-layout per-batch block in ht row (two 159B comp windows)
WN = 159  # new-layout window width


def build_nc(sgroups=None, groups=None, cgroups=None, fin_at=None,
             pbufs=(4, 4), psum_src=False, split_hankel=True,
             cast_eng=None, hank_eng=None, spal_eng="gpsimd",
             exp_eng=None, rows_eng=None, slay=None, tail_first=True):
    f8 = mybir.dt.float8e4
    f32 = mybir.dt.float32
    f32r = mybir.dt.float32r
    sgroups = list(SGROUPS if sgroups is None else sgroups)
    groups = list(GROUPS if groups is None else groups)
    cgs = list(CGROUPS if cgroups is None else cgroups)
    assert sum(sgroups) == BPC and sum(groups) == BPC and sum(cgs) == BPC
    sstart = [sum(sgroups[:i]) for i in range(len(sgroups))]
    gstart = [sum(groups[:i]) for i in range(len(groups))]
    cstart = [sum(cgs[:i]) for i in range(len(cgs))]
    ng = len(groups)
    if cast_eng is None:
        cast_eng = ("gpsimd",) * len(cgs)
    if hank_eng is None:
        hank_eng = [("sync", "sync"), ("sync", "sync"),
                    ("sync", "scalar"), ("sync", "scalar")][:len(sgroups)]
        while len(hank_eng) < len(sgroups):
            hank_eng.append(("sync", "scalar"))
    if exp_eng is None:
        exp_eng = ["sync", "sync", "sync", "scalar", "sync", "scalar",
                   "sync", "scalar", "sync"][:ng]
    if rows_eng is None:
        rows_eng = ["vector"] * ng
    if slay is None:
        slay = ["o"] * len(sgroups)
    # map batch -> supergroup index
    b2s = []
    for s, sz in enumerate(sgroups):
        b2s += [s] * sz
    # per cast group layout = layout of the sgs it covers (must be uniform)
    clay = []
    for ci in range(len(cgs)):
        lays = {
            slay[s]
            for s in range(len(sgroups))
            if cstart[ci] <= sstart[s] < cstart[ci] + cgs[ci]
        }
        assert len(lays) == 1, f"cast group {ci} mixes layouts: {lays}"
        clay.append(lays.pop())

    nc = bacc.Bacc(
        "TRN2", target_bir_lowering=False, debug=False, num_devices=NCORES
    )
    inp = nc.dram_tensor("inp", [BPC, 2, N], f32, kind="ExternalInput")
    spald = nc.dram_tensor("spald", [128, 256], f32r, kind="ExternalInput")
    out = nc.dram_tensor("out", [BPC, L, L], f32, kind="ExternalOutput")

    def eng(name):
        return getattr(nc, name)

    with tile.TileContext(nc) as tc:
        with (
            tc.tile_pool(name="const", bufs=1) as cpool,
            tc.tile_pool(name="dram", bufs=1, space="DRAM") as dpool,
            tc.tile_pool(name="hank", bufs=len(sgroups)) as hpool,
            tc.tile_pool(name="spal", bufs=ng) as spool,
            tc.tile_pool(name="rr", bufs=ng) as rpool,
            tc.tile_pool(name="psum", bufs=pbufs[0], space="PSUM") as ppool,
            tc.tile_pool(name="pst", bufs=pbufs[1], space="PSUM") as tpool,
        ):
            # --- doubled fp8 signal in DRAM via tapered casting DMAs,
            # emitted FIRST. Each cast group gets its own DRAM tile so
            # hankel DMAs depend only on their own cast. ---
            flat = inp[:].rearrange("b c n -> (b c) n")
            xdups = []  # one per cast group
            for si, ssz in enumerate(cgs):
                ce = eng(cast_eng[min(si, len(cast_eng) - 1)])
                r0 = 2 * cstart[si]
                if clay[si] == "o":
                    xd = dpool.tile([2 * ssz, CROW], f8)
                    src0 = AP(
                        tensor=flat.tensor,
                        offset=flat.offset + r0 * N,
                        ap=[[N, 2 * ssz], [0, 2], [1, N]],
                    )
                    dst0 = AP(
                        tensor=xd.tensor,
                        offset=xd.offset,
                        ap=[[CROW, 2 * ssz], [N, 2], [1, N]],
                    )
                    ce.dma_start(out=dst0, in_=src0)
                else:
                    # new layout: row per (b,c) = [x_c | x_c[0:128]]
                    xd = dpool.tile([2 * ssz, DXN], f8)

                    def _tail():
                        ce.dma_start(
                            out=AP(
                                tensor=xd.tensor,
                                offset=xd.offset + N,
                                ap=[[DXN, 2 * ssz], [1, 128]],
                            ),
                            in_=AP(
                                tensor=flat.tensor,
                                offset=flat.offset + r0 * N,
                                ap=[[N, 2 * ssz], [1, 128]],
                            ),
                        )

                    if tail_first:
                        _tail()
                    ce.dma_start(
                        out=AP(
                            tensor=xd.tensor,
                            offset=xd.offset,
                            ap=[[DXN, 2 * ssz], [1, N]],
                        ),
                        in_=AP(
                            tensor=flat.tensor,
                            offset=flat.offset + r0 * N,
                            ap=[[N, 2 * ssz], [1, N]],
                        ),
                    )
                    if not tail_first:
                        _tail()
                xdups.append(xd)

            # --- palindrome matrix for the finish matmul ---
            spal_t = cpool.tile([128, 256], f32r)
            eng(spal_eng).dma_start(out=spal_t[:], in_=spald[:])

            # --- hankel tiles ---
            # old: H[16c+p, j*W+u] = x_c[b_j, p+u]
            # new: H[j, b*318+159c+u] = x_c[b, 32j+u]  (32 shifts, pair=comp)
            htiles = []
            for s, ssz in enumerate(sgroups):
                ci = max(i for i in range(len(cgs)) if cstart[i] <= sstart[s])
                assert cstart[ci] + cgs[ci] >= sstart[s] + ssz, "sg spans casts"
                xd = xdups[ci]
                if slay[s] == "n":
                    ht = hpool.tile([32, ssz * BWN], f8)
                    htiles.append(ht)
                    he = hank_eng[s]
                    if isinstance(he, str):
                        he = (he, he)
                    xoffn = 2 * (sstart[s] - cstart[ci]) * DXN
                    for c in range(2):
                        src = AP(
                            tensor=xd.tensor,
                            offset=xd.offset + xoffn + c * DXN,
                            ap=[[32, 32], [2 * DXN, ssz], [1, WN]],
                        )
                        dst = AP(
                            tensor=ht.tensor,
                            offset=ht.offset + c * WN,
                            ap=[[ssz * BWN, 32], [BWN, ssz], [1, WN]],
                        )
                        eng(he[c]).dma_start(out=dst, in_=src)
                    continue
                ht = hpool.tile([K, ssz * W], f8)
                htiles.append(ht)
                xoff = 2 * (sstart[s] - cstart[ci]) * CROW
                W_ = ssz * W
                if split_hankel:
                    he = hank_eng[s]
                    if isinstance(he, str):
                        he = (he, he)
                    for c in range(2):
                        src = AP(
                            tensor=xd.tensor,
                            offset=xd.offset + xoff + c * CROW,
                            ap=[[1, P], [ROW, ssz], [1, W]],
                        )
                        eng(he[c]).dma_start(
                            out=ht[P * c : P * c + P, :], in_=src
                        )
                else:
                    src = AP(
                        tensor=xd.tensor,
                        offset=xd.offset + xoff,
                        ap=[[CROW, 2], [1, P], [ROW, ssz], [1, W]],
                    )
                    dst = AP(
                        tensor=ht.tensor,
                        offset=ht.offset,
                        ap=[[P * W_, 2], [W_, P], [W, ssz], [1, W]],
                    )
                    eng(hank_eng[s]).dma_start(out=dst, in_=src)

            rgs = {}

            def finish(g):
                gb = groups[g]
                # one matmul applies transpose+mirror+1/L:
                # pt2[j,k] = sum_d rg[d,j]*spal[d,k],  spal[d,k]=[d==|k-127|]/L
                pt2 = tpool.tile([gb, 256], f32)
                nc.tensor.matmul(pt2[:], rgs[g][:], spal_t[:])
                if psum_src:
                    src2 = AP(
                        tensor=pt2.tensor,
                        offset=pt2.offset + 127,
                        ap=[[256, gb], [-1, 128], [1, 128]],
                    )
                else:
                    rows = spool.tile([gb, 256], f32)
                    re = rows_eng[g]
                    if re == "scalar":
                        nc.scalar.mul(rows[:, 0:255], pt2[:, 0:255], 1.0)
                    elif re == "split":
                        nc.vector.tensor_copy(rows[:, 0:128], pt2[:, 0:128])
                        nc.scalar.mul(rows[:, 128:255], pt2[:, 128:255], 1.0)
                    else:
                        nc.vector.tensor_copy(rows[:, 0:255], pt2[:, 0:255])
                    src2 = AP(
                        tensor=rows.tensor,
                        offset=rows.offset + 127,
                        ap=[[256, gb], [-1, 128], [1, 128]],
                    )
                dst2 = AP(
                    tensor=out,
                    offset=gstart[g] * L * L,
                    ap=[[L * L, gb], [L, 128], [1, 128]],
                )
                eng(exp_eng[g]).dma_start(out=dst2, in_=src2)

            for g in range(ng):
                gb = groups[g]
                ps = ppool.tile([128, gb], f32)
                fa = fin_at if fin_at is not None else max(1, gb // 2)
                for j in range(gb):
                    if j == min(fa, gb - 1) and g >= 1:
                        finish(g - 1)
                    b = gstart[g] + j
                    s = b2s[b]
                    ht = htiles[s]
                    if slay[s] == "n":
                        col = (b - sstart[s]) * BWN
                        prow = sgroups[s] * BWN
                        for tp in range(32):
                            off = col + tp
                            lhsT = AP(
                                tensor=ht.tensor,
                                offset=ht.offset + off,
                                ap=[[prow, 32], [WN, 2], [1, 128]],
                            )
                            rhs = AP(
                                tensor=ht.tensor,
                                offset=ht.offset + off,
                                ap=[[prow, 32], [WN, 2], [1, 1]],
                            )
                            nc.tensor.matmul(
                                ps[:, j : j + 1],
                                lhsT,
                                rhs,
                                start=(tp == 0),
                                stop=(tp == 31),
                                perf_mode=mybir.MatmulPerfMode.DoubleRow,
                            )
                        continue
                    col = (b - sstart[s]) * W
                    for tp in range(T // 2):
                        off = col + K * tp
                        lhsT = AP(
                            tensor=ht.tensor,
                            offset=ht.offset + off,
                            ap=[[sgroups[s] * W, K], [P, 2], [1, 128]],
                        )
                        rhs = AP(
                            tensor=ht.tensor,
                            offset=ht.offset + off,
                            ap=[[sgroups[s] * W, K], [P, 2], [1, 1]],
                        )
                        nc.tensor.matmul(
                            ps[:, j : j + 1],
                            lhsT,
                            rhs,
                            start=(tp == 0),
                            stop=(tp == T // 2 - 1),
                            perf_mode=mybir.MatmulPerfMode.DoubleRow,
                        )

                rg = rpool.tile([128, gb], f32r)
                nc.vector.tensor_copy(rg[:], ps[:])
                rgs[g] = rg
            finish(ng - 1)

    nc.compile()
    return nc


def kernel(inputs: np.ndarray) -> np.ndarray:
    global LAST_RESULT
    inputs = np.ascontiguousarray(np.asarray(inputs), dtype=np.float32)
    assert inputs.shape == (B, 2, N), inputs.shape

    if "nc" not in _CACHE:
        _CACHE["nc"] = build_nc()
    nc = _CACHE["nc"]

    k = np.arange(256)
    d = np.arange(128)
    spal = (d[:, None] == np.minimum(np.abs(k[None, :] - 127), 127)).astype(
        np.float32
    ) / L
    spal[:, 255] = 0.0
    in_maps = [
        {"inp": inputs[c * BPC : (c + 1) * BPC], "spald": spal}
        for c in range(NCORES)
    ]
    res = run_bass_kernel_spmd(nc, in_maps, list(range(NCORES)), trace=False)
    LAST_RESULT = res
    outf = np.concatenate([res.results[c]["out"] for c in range(NCORES)], axis=0)
    return outf.reshape(B, L, L, 1).astype(np.float32, copy=False)
